# revision 9
# baseline (speedup 1.0000x reference)
"""DiffuserSelfAttention (sparse attention) Trainium2 Bass kernel.

Strategy: the edge-list graph attention is reformulated as dense masked
attention (density ~35%), head-parallel across the 8 NeuronCores (NH=8
heads, one head per core, zero collectives).

Per core (head h):
  1. qkT [128,1024] = [Wq_h/8 | Wk_h] @ hsT   (bias folded via ones-row)
  2. v   [1024,64]  (normal layout, i on partitions)
  3. St[j,i] = sum_d kT[d,j] qT[d,i]          (PE, K=64)
  4. Wt = exp(St) * adjmask                   (ScalarE exp + VectorE mul)
  5. 5 rounds: h <- 0.9 * (Wt^T h)/denom + 0.1 v ; denom from a ones
     column appended to h in round 0 (exact softmax denominator).

All matmuls in bf16 (measured end-to-end rel err ~2.4e-3 vs f32 ref).

Self-contained: hardcodes B=1, S=1024, HIDDEN=512, NH=8, HD=64.
"""

import numpy as np
import ml_dtypes

S = 1024
HIDDEN = 512
NH = 8
HD = 64
P = 128
NT_S = S // P            # 8 node tiles
KDIM = HIDDEN + P        # 640: hidden + ones-row (bias) + zero pad
NT_K = KDIM // P         # 5 contraction tiles for projections
ALPHA = 0.1
N_ROUNDS = 5

_CACHED = {}


def _build_module():
    import concourse.bass as bass
    import concourse.tile as tile
    from concourse import bacc
    import concourse.mybir as mybir

    f32 = mybir.dt.float32
    bf16 = mybir.dt.bfloat16
    AF = mybir.ActivationFunctionType

    nc = bacc.Bacc("TRN2", target_bir_lowering=False, debug=False, num_devices=NH)

    hsT_d = nc.dram_tensor("hsT", [KDIM, S], bf16, kind="ExternalInput")
    wqk_d = nc.dram_tensor("wqk", [KDIM, P], bf16, kind="ExternalInput")
    wv_d = nc.dram_tensor("wv", [KDIM, HD], bf16, kind="ExternalInput")
    adjT_d = nc.dram_tensor("adjT", [S, S], bf16, kind="ExternalInput")
    out_d = nc.dram_tensor("out", [S, HD], f32, kind="ExternalOutput")

    with tile.TileContext(nc) as tc:
        with (
            tc.tile_pool(name="singles", bufs=1) as singles,
            tc.tile_pool(name="work", bufs=3) as work,
            tc.tile_pool(name="psum_big", bufs=2, space="PSUM") as psum_big,
            tc.tile_pool(name="psum_small", bufs=3, space="PSUM") as psum_small,
        ):
            # ---- load inputs ----
            hsT_sb = singles.tile([P, NT_K, S], bf16)
            nc.sync.dma_start(hsT_sb[:], hsT_d.ap().rearrange("(ko p) i -> p ko i", p=P))
            wqk_sb = singles.tile([P, NT_K, P], bf16)
            nc.sync.dma_start(wqk_sb[:], wqk_d.ap().rearrange("(ko p) m -> p ko m", p=P))
            wv_sb = singles.tile([P, NT_K, HD], bf16)
            nc.sync.dma_start(wv_sb[:], wv_d.ap().rearrange("(ko p) m -> p ko m", p=P))
            adjT_sb = singles.tile([P, NT_S, S], bf16)
            nc.sync.dma_start(adjT_sb[:], adjT_d.ap().rearrange("(t p) i -> p t i", p=P))

            # ---- persistent intermediates ----
            qT_sb = singles.tile([HD, S], bf16)
            kT_sb = singles.tile([HD, S], bf16)
            wt_sb = singles.tile([P, NT_S, S], bf16)     # masked exp(score), [j, i]
            h0_sb = singles.tile([P, NT_S, HD + 1], bf16)  # v with ones column
            av_sb = singles.tile([P, NT_S, HD], f32)     # 0.1 * v
            h_a = singles.tile([P, NT_S, HD], bf16)
            h_b = singles.tile([P, NT_S, HD], bf16)
            recip_sb = singles.tile([P, NT_S, 1], f32)   # 0.9 / denom per i
            out_sb = singles.tile([P, NT_S, HD], f32)

            # ---- stage 1: qT/kT = w.T @ hsT -> [64, 1024] each ----
            for w0, dst, nm in ((0, qT_sb, "q"), (HD, kT_sb, "k")):
                ps_qk = psum_big.tile([HD, S], f32, name=f"ps_{nm}", tag="ps_big")
                for n in range(2):
                    for ke in range(NT_K):
                        nc.tensor.matmul(
                            ps_qk[:, bass.ts(n, 512)],
                            wqk_sb[:, ke, w0 : w0 + HD],
                            hsT_sb[:, ke, bass.ts(n, 512)],
                            start=(ke == 0),
                            stop=(ke == NT_K - 1),
                        )
                nc.scalar.activation(out=dst[:], in_=ps_qk[:], func=AF.Copy)

            # ---- stage 2: v (normal layout) + ones column + av ----
            # 4 i-tiles packed per PSUM bank so the PSUM->SBUF copies batch.
            nc.vector.memset(h0_sb[:, :, HD : HD + 1], 1.0)
            for ih in range(2):
                ps_v = psum_small.tile([P, 4, HD], f32, name=f"ps_v{ih}", tag="ps_small")
                for il in range(4):
                    it = ih * 4 + il
                    for ke in range(NT_K):
                        nc.tensor.matmul(
                            ps_v[:, il, :],
                            hsT_sb[:, ke, bass.ts(it, P)],
                            wv_sb[:, ke, :],
                            start=(ke == 0),
                            stop=(ke == NT_K - 1),
                        )
                sl = slice(ih * 4, ih * 4 + 4)
                nc.scalar.activation(out=h0_sb[:, sl, :HD], in_=ps_v[:], func=AF.Copy)
                nc.scalar.activation(
                    out=av_sb[:, sl, :], in_=ps_v[:], func=AF.Copy, scale=ALPHA
                )

            # ---- stage 3: scores + exp + mask (whole j-tile rows at once) ----
            for jt in range(NT_S):
                ps_s = psum_big.tile([P, S], f32, name=f"ps_s{jt}", tag="ps_big")
                for n in range(2):
                    nc.tensor.matmul(
                        ps_s[:, bass.ts(n, 512)],
                        kT_sb[:, bass.ts(jt, P)],
                        qT_sb[:, bass.ts(n, 512)],
                        start=True,
                        stop=True,
                    )
                exp_t = work.tile([P, S], bf16, name="exp_t", tag="exp_t")
                nc.scalar.activation(out=exp_t[:], in_=ps_s[:], func=AF.Exp)
                nc.vector.tensor_mul(
                    out=wt_sb[:, jt, :], in0=exp_t[:], in1=adjT_sb[:, jt, :]
                )

            # ---- stage 4: propagation rounds (batched normalize+residual) ----
            for r in range(N_ROUNDS):
                if r == 0:
                    h_cur, ncols = h0_sb, HD + 1
                elif r % 2 == 1:
                    h_cur, ncols = h_a, HD
                else:
                    h_cur, ncols = h_b, HD
                h_next = h_b if r % 2 == 1 else h_a
                dst = h_next if r < N_ROUNDS - 1 else out_sb
                nhalf = 2 if r == 0 else 1
                for ih in range(nhalf):
                    nit = NT_S // nhalf
                    ps_p = psum_small.tile(
                        [P, nit, ncols], f32, name=f"ps_p{r}_{ih}", tag="ps_small"
                    )
                    for il in range(nit):
                        it = ih * nit + il
                        for jt in range(NT_S):
                            nc.tensor.matmul(
                                ps_p[:, il, :],
                                wt_sb[:, jt, bass.ts(it, P)],
                                h_cur[:, jt, :ncols],
                                start=(jt == 0),
                                stop=(jt == NT_S - 1),
                            )
                    sl = slice(ih * nit, ih * nit + nit)
                    if r == 0:
                        # recip = 0.9 / denom  (denom lives in the ones column)
                        den_t = work.tile([P, nit, 1], f32, name="den_t", tag="den_t")
                        nc.vector.tensor_scalar_mul(
                            den_t[:], ps_p[:, :, HD : HD + 1], 1.0 / (1.0 - ALPHA)
                        )
                        nc.vector.reciprocal(recip_sb[:, sl, :], den_t[:])
                    tmp = work.tile([P, nit, HD], f32, name="tmp_sc", tag="tmp_sc")
                    nc.vector.tensor_mul(
                        out=tmp[:],
                        in0=ps_p[:, :, :HD],
                        in1=recip_sb[:, sl, :].to_broadcast([P, nit, HD]),
                    )
                    nc.vector.tensor_add(
                        out=dst[:, sl, :], in0=tmp[:], in1=av_sb[:, sl, :]
                    )

            nc.sync.dma_start(out_d.ap().rearrange("(t p) d -> p t d", p=P), out_sb[:])

    nc.compile()
    return nc


def _prep_inputs(hidden_states, attention_mask, Wq, bq, Wk, bk, Wv, bv, src, dst):
    bf = ml_dtypes.bfloat16
    hs = np.asarray(hidden_states, np.float32).reshape(S, HIDDEN)
    scale = 1.0 / np.sqrt(HD)

    hsT = np.zeros((KDIM, S), np.float32)
    hsT[:HIDDEN] = hs.T
    hsT[HIDDEN] = 1.0  # bias row
    hsT = hsT.astype(bf)

    WqT = np.asarray(Wq, np.float32).T * scale  # [HIDDEN, HIDDEN]
    WkT = np.asarray(Wk, np.float32).T
    WvT = np.asarray(Wv, np.float32).T
    bq_s = np.asarray(bq, np.float32) * scale
    bk_ = np.asarray(bk, np.float32)
    bv_ = np.asarray(bv, np.float32)

    # dense adjacency in [src, dst] layout, combined with the attention mask
    ok = (np.asarray(attention_mask, np.float32).reshape(S) > 0)
    adjT = np.zeros((S, S), np.float32)
    adjT[np.asarray(src), np.asarray(dst)] = 1.0
    adjT *= ok[:, None]
    adjT *= ok[None, :]
    adjT = adjT.astype(bf)

    in_maps = []
    for h in range(NH):
        sl = slice(h * HD, (h + 1) * HD)
        wqk = np.zeros((KDIM, P), np.float32)
        wqk[:HIDDEN, :HD] = WqT[:, sl]
        wqk[:HIDDEN, HD:] = WkT[:, sl]
        wqk[HIDDEN, :HD] = bq_s[sl]
        wqk[HIDDEN, HD:] = bk_[sl]
        wv = np.zeros((KDIM, HD), np.float32)
        wv[:HIDDEN] = WvT[:, sl]
        wv[HIDDEN] = bv_[sl]
        in_maps.append(
            {
                "hsT": hsT,
                "wqk": wqk.astype(bf),
                "wv": wv.astype(bf),
                "adjT": adjT,
            }
        )
    return in_maps


def kernel(**inputs):
    from concourse.bass_utils import run_bass_kernel_spmd

    if "nc" not in _CACHED:
        _CACHED["nc"] = _build_module()
    nc = _CACHED["nc"]

    in_maps = _prep_inputs(**inputs)
    import os

    trace = bool(int(os.environ.get("KERNEL_TRACE", "0")))
    res = run_bass_kernel_spmd(
        nc,
        in_maps,
        core_ids=list(range(NH)),
        trace=trace,
        trace_cores=list(range(NH)) if trace else None,
    )
    _CACHED["last_results"] = res

    out = np.concatenate([res.results[h]["out"] for h in range(NH)], axis=1)
    return out.reshape(1, S, NH * HD).astype(np.float32)


# revision 12
# speedup vs baseline: 1.0814x; 1.0814x over previous
"""DiffuserSelfAttention (sparse attention) Trainium2 Bass kernel.

Strategy: the edge-list graph attention is reformulated as dense masked
attention (density ~35%), head-parallel across the 8 NeuronCores (NH=8
heads, one head per core, zero collectives).

Per core (head h):
  1. qkT = [Wq_h/8 | Wk_h] @ hsT  (biases folded in via a ones-row)
  2. v [1024,64] (normal layout, i on partitions)
  3. St[j,i] = sum_d kT[d,j] qT[d,i]     (PE, K=64)
  4. Wt = exp(St) * adjmask              (ScalarE exp + VectorE mul)
  5. 5 rounds: h <- 0.9 * (Wt^T h)/denom + 0.1 v ; denom comes from a
     ones column appended to h in round 0 (exact softmax denominator).
     Round 0 is interleaved with the score/exp pipeline per j-tile.

A PE warmup burst of dummy matmuls runs during the input-DMA window so
the HAM clock gate reaches 2.4 GHz before real work starts.

All matmuls in bf16 (measured end-to-end rel err ~2.4e-3 vs f32 ref).

Self-contained: hardcodes B=1, S=1024, HIDDEN=512, NH=8, HD=64.
"""

import numpy as np
import ml_dtypes

S = 1024
HIDDEN = 512
NH = 8
HD = 64
P = 128
NT_S = S // P            # 8 node tiles
KDIM = HIDDEN + P        # 640: hidden + ones-row (bias) + zero pad
NT_K = KDIM // P         # 5 contraction tiles for projections
ALPHA = 0.1
N_ROUNDS = 5
import os as _os
WARMUP_MMS = int(_os.environ.get("WARMUP_MMS", "24"))
ROUND0_INTERLEAVE = bool(int(_os.environ.get("ROUND0_INTERLEAVE", "1")))

_CACHED = {}


def _build_module():
    import concourse.bass as bass
    import concourse.tile as tile
    from concourse import bacc
    import concourse.mybir as mybir

    f32 = mybir.dt.float32
    bf16 = mybir.dt.bfloat16
    AF = mybir.ActivationFunctionType
    ts = bass.ts

    nc = bacc.Bacc("TRN2", target_bir_lowering=False, debug=False, num_devices=NH)

    hsT_d = nc.dram_tensor("hsT", [KDIM, S], bf16, kind="ExternalInput")
    wqk_d = nc.dram_tensor("wqk", [KDIM, P], bf16, kind="ExternalInput")
    wv_d = nc.dram_tensor("wv", [KDIM, HD], bf16, kind="ExternalInput")
    adjT_d = nc.dram_tensor("adjT", [S, S], bf16, kind="ExternalInput")
    out_d = nc.dram_tensor("out", [S, HD], f32, kind="ExternalOutput")

    hsT_t = hsT_d.ap().rearrange("(ko p) i -> p ko i", p=P)
    adjT_t = adjT_d.ap().rearrange("(t p) i -> p t i", p=P)
    out_t = out_d.ap().rearrange("(t p) d -> p t d", p=P)

    with tile.TileContext(nc) as tc:
        with (
            tc.tile_pool(name="singles", bufs=1) as singles,
            tc.tile_pool(name="work", bufs=3) as work,
            tc.tile_pool(name="psum_warm", bufs=1, space="PSUM") as psum_warm,
            tc.tile_pool(name="psum_big", bufs=2, space="PSUM") as psum_big,
            tc.tile_pool(name="psum_small", bufs=3, space="PSUM") as psum_small,
        ):
            # ---- PE warmup: dummy matmuls on scratch while inputs DMA in ----
            scratch = singles.tile([P, 512], bf16)
            nc.vector.memset(scratch[:], 0.0)
            ps_w = psum_warm.tile([P, 512], f32)
            for _ in range(WARMUP_MMS):
                nc.tensor.matmul(
                    ps_w[:], scratch[:, :P], scratch[:], start=True, stop=True
                )

            # ---- load inputs (small weights first, then hsT, then adjT) ----
            wqk_sb = singles.tile([P, NT_K, P], bf16)
            nc.sync.dma_start(wqk_sb[:], wqk_d.ap().rearrange("(ko p) m -> p ko m", p=P))
            wv_sb = singles.tile([P, NT_K, HD], bf16)
            nc.sync.dma_start(wv_sb[:], wv_d.ap().rearrange("(ko p) m -> p ko m", p=P))
            hsT_sb = singles.tile([P, NT_K, S], bf16)
            for ke in range(NT_K):
                nc.sync.dma_start(hsT_sb[:, ke, :], hsT_t[:, ke, :])
            adjT_sb = singles.tile([P, NT_S, S], bf16)
            for jt in range(NT_S):
                nc.sync.dma_start(adjT_sb[:, jt, :], adjT_t[:, jt, :])

            # ---- persistent intermediates ----
            qT_sb = singles.tile([HD, S], bf16)
            kT_sb = singles.tile([HD, S], bf16)
            wt_sb = singles.tile([P, NT_S, S], bf16)       # masked exp(score), [j, i]
            h0_sb = singles.tile([P, NT_S, HD + 1], bf16)  # v with ones column
            av_sb = singles.tile([P, NT_S, HD], f32)       # 0.1 * v
            h_a = singles.tile([P, NT_S, HD], bf16)
            h_b = singles.tile([P, NT_S, HD], bf16)
            recip_sb = singles.tile([P, NT_S, 1], f32)     # 0.9 / denom per i
            out_sb = singles.tile([P, NT_S, HD], f32)

            # ---- stage 1: v (ke-major so it can start on the first hsT tile) ----
            nc.vector.memset(h0_sb[:, :, HD : HD + 1], 1.0)
            ps_vs = []
            for ih in range(2):
                ps_v = psum_small.tile([P, 4, HD], f32, name=f"ps_v{ih}", tag="ps_small")
                ps_vs.append(ps_v)
            # NOTE: accumulation groups sharing a PSUM bank must be
            # sequential — start=True clears group state bank-wide, so
            # interleaving groups drops contributions.
            for it in range(NT_S):
                for ke in range(NT_K):
                    nc.tensor.matmul(
                        ps_vs[it // 4][:, it % 4, :],
                        hsT_sb[:, ke, ts(it, P)],
                        wv_sb[:, ke, :],
                        start=(ke == 0),
                        stop=(ke == NT_K - 1),
                    )
            for ih in range(2):
                sl = slice(ih * 4, ih * 4 + 4)
                nc.scalar.activation(out=h0_sb[:, sl, :HD], in_=ps_vs[ih][:], func=AF.Copy)
                nc.vector.tensor_scalar_mul(av_sb[:, sl, :], ps_vs[ih][:], ALPHA)

            # ---- stage 2: qT/kT ----
            for w0, dst, nm in ((0, qT_sb, "q"), (HD, kT_sb, "k")):
                ps_qk = psum_big.tile([HD, S], f32, name=f"ps_{nm}", tag="ps_big")
                for n in range(2):
                    for ke in range(NT_K):
                        nc.tensor.matmul(
                            ps_qk[:, ts(n, 512)],
                            wqk_sb[:, ke, w0 : w0 + HD],
                            hsT_sb[:, ke, ts(n, 512)],
                            start=(ke == 0),
                            stop=(ke == NT_K - 1),
                        )
                nc.scalar.activation(out=dst[:], in_=ps_qk[:], func=AF.Copy)

            # ---- stage 3+round0: per j-tile score -> exp -> mask -> accumulate ----
            ps_p0 = []
            for ih in range(2):
                ps = psum_small.tile(
                    [P, 4, HD + 1], f32, name=f"ps_p0_{ih}", tag="ps_small"
                )
                ps_p0.append(ps)
            for jt in range(NT_S):
                ps_s = psum_big.tile([P, S], f32, name=f"ps_s{jt}", tag="ps_big")
                for n in range(2):
                    nc.tensor.matmul(
                        ps_s[:, ts(n, 512)],
                        kT_sb[:, ts(jt, P)],
                        qT_sb[:, ts(n, 512)],
                        start=True,
                        stop=True,
                    )
                exp_t = work.tile([P, S], bf16, name="exp_t", tag="exp_t")
                nc.scalar.activation(out=exp_t[:], in_=ps_s[:], func=AF.Exp)
                nc.vector.tensor_mul(
                    out=wt_sb[:, jt, :], in0=exp_t[:], in1=adjT_sb[:, jt, :]
                )
                if ROUND0_INTERLEAVE:
                    # round-0 contribution of this j-tile to every i-tile
                    for it in range(NT_S):
                        nc.tensor.matmul(
                            ps_p0[it // 4][:, it % 4, :],
                            wt_sb[:, jt, ts(it, P)],
                            h0_sb[:, jt, :],
                            start=(jt == 0),
                            stop=(jt == NT_S - 1),
                        )

            if not ROUND0_INTERLEAVE:
                for it in range(NT_S):
                    for jt in range(NT_S):
                        nc.tensor.matmul(
                            ps_p0[it // 4][:, it % 4, :],
                            wt_sb[:, jt, ts(it, P)],
                            h0_sb[:, jt, :],
                            start=(jt == 0),
                            stop=(jt == NT_S - 1),
                        )

            def finish_round(ps, ih, r, dst):
                """normalize + residual for one half of the i-tiles"""
                sl = slice(ih * 4, ih * 4 + 4)
                if r == 0:
                    den_t = work.tile([P, 4, 1], f32, name="den_t", tag="den_t")
                    nc.vector.tensor_scalar_mul(
                        den_t[:], ps[:, :, HD : HD + 1], 1.0 / (1.0 - ALPHA)
                    )
                    nc.vector.reciprocal(recip_sb[:, sl, :], den_t[:])
                tmp = work.tile([P, 4, HD], f32, name="tmp_sc", tag="tmp_sc")
                nc.vector.tensor_mul(
                    out=tmp[:],
                    in0=ps[:, :, :HD],
                    in1=recip_sb[:, sl, :].to_broadcast([P, 4, HD]),
                )
                nc.vector.tensor_add(out=dst[:, sl, :], in0=tmp[:], in1=av_sb[:, sl, :])

            for ih in range(2):
                finish_round(ps_p0[ih], ih, 0, h_a)

            # ---- rounds 1..4 ----
            for r in range(1, N_ROUNDS):
                h_cur = h_a if r % 2 == 1 else h_b
                h_next = h_b if r % 2 == 1 else h_a
                dst = h_next if r < N_ROUNDS - 1 else out_sb
                for ih in range(2):
                    ps_p = psum_small.tile(
                        [P, 4, HD], f32, name=f"ps_p{r}_{ih}", tag="ps_small"
                    )
                    for il in range(4):
                        it = ih * 4 + il
                        for jt in range(NT_S):
                            nc.tensor.matmul(
                                ps_p[:, il, :],
                                wt_sb[:, jt, ts(it, P)],
                                h_cur[:, jt, :],
                                start=(jt == 0),
                                stop=(jt == NT_S - 1),
                            )
                    finish_round(ps_p, ih, r, dst)
                    if r == N_ROUNDS - 1:
                        sl = slice(ih * 4, ih * 4 + 4)
                        nc.sync.dma_start(out_t[:, sl, :], out_sb[:, sl, :])

    nc.compile()
    return nc


def _prep_inputs(hidden_states, attention_mask, Wq, bq, Wk, bk, Wv, bv, src, dst):
    bf = ml_dtypes.bfloat16
    hs = np.asarray(hidden_states, np.float32).reshape(S, HIDDEN)
    scale = 1.0 / np.sqrt(HD)

    hsT = np.zeros((KDIM, S), np.float32)
    hsT[:HIDDEN] = hs.T
    hsT[HIDDEN] = 1.0  # bias row
    hsT = hsT.astype(bf)

    WqT = np.asarray(Wq, np.float32).T * scale  # [HIDDEN, HIDDEN]
    WkT = np.asarray(Wk, np.float32).T
    WvT = np.asarray(Wv, np.float32).T
    bq_s = np.asarray(bq, np.float32) * scale
    bk_ = np.asarray(bk, np.float32)
    bv_ = np.asarray(bv, np.float32)

    # dense adjacency in [src, dst] layout, combined with the attention mask
    ok = (np.asarray(attention_mask, np.float32).reshape(S) > 0)
    adjT = np.zeros((S, S), np.float32)
    adjT[np.asarray(src), np.asarray(dst)] = 1.0
    adjT *= ok[:, None]
    adjT *= ok[None, :]
    adjT = adjT.astype(bf)

    in_maps = []
    for h in range(NH):
        sl = slice(h * HD, (h + 1) * HD)
        wqk = np.zeros((KDIM, P), np.float32)
        wqk[:HIDDEN, :HD] = WqT[:, sl]
        wqk[:HIDDEN, HD:] = WkT[:, sl]
        wqk[HIDDEN, :HD] = bq_s[sl]
        wqk[HIDDEN, HD:] = bk_[sl]
        wv = np.zeros((KDIM, HD), np.float32)
        wv[:HIDDEN] = WvT[:, sl]
        wv[HIDDEN] = bv_[sl]
        in_maps.append(
            {
                "hsT": hsT,
                "wqk": wqk.astype(bf),
                "wv": wv.astype(bf),
                "adjT": adjT,
            }
        )
    return in_maps


def kernel(**inputs):
    from concourse.bass_utils import run_bass_kernel_spmd

    if "nc" not in _CACHED:
        _CACHED["nc"] = _build_module()
    nc = _CACHED["nc"]

    in_maps = _prep_inputs(**inputs)
    import os

    trace = bool(int(os.environ.get("KERNEL_TRACE", "0")))
    res = run_bass_kernel_spmd(
        nc,
        in_maps,
        core_ids=list(range(NH)),
        trace=trace,
        trace_cores=list(range(NH)) if trace else None,
    )
    _CACHED["last_results"] = res

    out = np.concatenate([res.results[h]["out"] for h in range(NH)], axis=1)
    return out.reshape(1, S, NH * HD).astype(np.float32)
